# revision 59
# baseline (speedup 1.0000x reference)
"""Multi-head causal attention (Whisper-style) on 8 trn2 NeuronCores.

Sharding: head-parallel (2 of 16 heads per core) + row-parallel output
projection.  Each core receives the full (host-transposed) activations
x^T (bf16), its column slice of Wq/Wk/Wv (+bias slices) and its row
slice of Wo, and produces a full-size partial y^T (bf16).  The host
sums the 8 partials, transposes back, and adds bo.

On-chip layout is fully "transposed" (feature dim on partitions):
  q^T,k^T = W^T-stationary matmuls over x^T -> f32r [128, S] per batch
  scores^T[k,q] per (batch, head) in [128,2,512] PSUM pairs, exp with
  no max-subtraction, softmax denominator via ones-columns in v.
Causal masking needs no mask DMA: blocks fully above the diagonal are
skipped, fully-masked column spans are never exp'd nor streamed through
the o^T matmul (sub-span accumulate), and the single 128x128 diagonal
sub-block is exp'd then multiplied by one shared 0/1 upper-triangular
bf16 tile.  Work from the three stages is interleaved (inject lists +
drains deferred across half boundaries) to keep the PE array busy.

Engine busy per core (cost-model estimate, ~170us wall):
  PE   ~124us  (projections 41 + transposes 3 + attention 63 + out-proj 14)
  ACT  ~94us   (exp + half the out-proj PSUM->SBUF drains)
  DVE  ~81us   (proj finalizes, tri-mults, softmax normalize, drains)
  DMA  ~55us   (x 23 + y 23 + weights/bcast); y issue via Pool SWDGE
"""

import os
import sys
from contextlib import ExitStack

import numpy as np

for _p in ("/root/.axon_site/_ro/trn_rl_repo", "/opt/trn_rl_repo"):
    if os.path.isdir(_p) and _p not in sys.path:
        sys.path.append(_p)

import concourse.bass as bass
import concourse.mybir as mybir
import concourse.tile as tile
from concourse import bacc, bass_utils

F32 = mybir.dt.float32
F32R = mybir.dt.float32r
BF16 = mybir.dt.bfloat16
AF = mybir.ActivationFunctionType
ALU = mybir.AluOpType

N_STATE = 1024
N_HEAD = 16
HD = 64
N_CORES = 8
HEADS_PER_CORE = N_HEAD // N_CORES  # 2
E = HEADS_PER_CORE * HD  # 128 feature columns per core
Q_TILE = 512
K_CHUNK = 128
G_TILE = 1024  # stage A/C token-group (2 q-tiles)
N_D = N_STATE // 128  # 8 contraction chunks for the projections
SCALE = float(HD) ** -0.25
NEG_THRESH = -50.0


def classify_blocks(maskT):
    """Value-driven classification of (k_chunk, q_tile) mask blocks.

    partial blocks additionally verify the shifted-triangular pattern
    (unmasked iff k <= q) that the on-device masking assumes.
    """
    S = maskT.shape[0]
    cls = {}
    for ki in range(S // K_CHUNK):
        for j in range(S // Q_TILE):
            blk = maskT[ki * K_CHUNK:(ki + 1) * K_CHUNK,
                        j * Q_TILE:(j + 1) * Q_TILE]
            if np.all(blk < NEG_THRESH):
                cls[(ki, j)] = "skip"
            elif np.all(blk == 0.0):
                cls[(ki, j)] = "clean"
            else:
                cls[(ki, j)] = "partial"
                k_idx = ki * K_CHUNK + np.arange(K_CHUNK)[:, None]
                q_idx = j * Q_TILE + np.arange(Q_TILE)[None, :]
                ok = (blk > NEG_THRESH) == (k_idx <= q_idx)
                assert ok.all(), f"partial block {(ki, j)} is not causal-triangular"
    return cls


def build_kernel(B, S, cls, repeats=1, only=None):
    """Build the per-core SPMD Bass program (identical on all cores)."""
    n_k = S // K_CHUNK
    n_q = S // Q_TILE
    n_g = S // G_TILE  # token groups per batch

    nc = bacc.Bacc("TRN2", target_bir_lowering=False, debug=False,
                   num_devices=N_CORES)

    xT_d = nc.dram_tensor("xT", [B, N_STATE, S], BF16, kind="ExternalInput")
    tri_d = nc.dram_tensor("tri", [K_CHUNK, K_CHUNK], BF16, kind="ExternalInput")
    wq_d = nc.dram_tensor("wq", [N_STATE, E], BF16, kind="ExternalInput")
    wk_d = nc.dram_tensor("wk", [N_STATE, E], BF16, kind="ExternalInput")
    wv_d = nc.dram_tensor("wv", [N_STATE, E], BF16, kind="ExternalInput")
    wo_d = nc.dram_tensor("wo", [E, N_STATE], BF16, kind="ExternalInput")
    bq_d = nc.dram_tensor("bq", [E], F32, kind="ExternalInput")
    ident_d = nc.dram_tensor("ident", [128, 128], BF16, kind="ExternalInput")
    bv_d = nc.dram_tensor("bv", [E], F32, kind="ExternalInput")
    yT_d = nc.dram_tensor("yT", [B, N_STATE, S], BF16, kind="ExternalOutput")

    # last contributing k-chunk per q-tile (for o^T stop flags)
    last_ki = {j: max(ki for ki in range(n_k) if cls[(ki, j)] != "skip")
               for j in range(n_q)}

    with tile.TileContext(nc) as tc, ExitStack() as ctx:
        const = ctx.enter_context(tc.tile_pool(name="const", bufs=1))
        xpool = ctx.enter_context(tc.tile_pool(name="xpool", bufs=2))
        vstage = ctx.enter_context(tc.tile_pool(name="vstage", bufs=2))
        wexp = ctx.enter_context(tc.tile_pool(name="wexp", bufs=6))
        otsb = ctx.enter_context(tc.tile_pool(name="otsb", bufs=4))
        nrm = ctx.enter_context(tc.tile_pool(name="nrm", bufs=6))
        yspool = ctx.enter_context(tc.tile_pool(name="yspool", bufs=6))
        # PSUM: psM 2x[128,2,512]f32 (4 banks) + psO 1x[65,2,512]f32 (2)
        # + psC 1x[128,2,512]f32 (2) = 8 banks.  C-units get their own slot
        # so a Pool-side backlog can't stall the scores-tile rotation.
        psM = ctx.enter_context(tc.tile_pool(name="psM", bufs=2, space="PSUM"))
        psO = ctx.enter_context(tc.tile_pool(name="psO", bufs=1, space="PSUM"))
        psC = ctx.enter_context(tc.tile_pool(name="psC", bufs=1, space="PSUM"))

        # ---- resident constants / weights (emitted in first-use order) ----
        wq_sb = const.tile([128, N_D, E], BF16, tag="wq_sb")
        wk_sb = const.tile([128, N_D, E], BF16, tag="wk_sb")
        wv_sb = const.tile([128, N_D, E], BF16, tag="wv_sb")
        nc.sync.dma_start(wq_sb[:], wq_d[:].rearrange("(c p) e -> p c e", p=128))
        bq_sb = const.tile([E, 1], F32, tag="bq_sb")
        bv_sb = const.tile([E, 1], F32, tag="bv_sb")
        tri = const.tile([K_CHUNK, K_CHUNK], BF16, tag="tri")
        ident = const.tile([128, 128], BF16, tag="ident")

        qT = [const.tile([E, S], F32R, name=f"qT{b}", tag=f"qT{b}")
              for b in range(B)]
        kT = [const.tile([E, S], F32R, name=f"kT{b}", tag=f"kT{b}")
              for b in range(B)]
        onT = [const.tile([E, S], BF16, name=f"onT{b}", tag=f"onT{b}")
               for b in range(B)]
        # v + ones columns: [h0 64 | 1 | h1 64 | 1] per k-chunk
        vn = [const.tile([128, n_k, 2 * (HD + 1)], BF16, name=f"vn{b}",
                         tag=f"vn{b}") for b in range(B)]

        wo_sb = None

        def load_rest_consts():
            nonlocal wo_sb
            nc.sync.dma_start(wk_sb[:], wk_d[:].rearrange("(c p) e -> p c e", p=128))
            nc.sync.dma_start(wv_sb[:], wv_d[:].rearrange("(c p) e -> p c e", p=128))
            nc.sync.dma_start(bq_sb[:], bq_d[:].rearrange("(e o) -> e o", o=1))
            nc.sync.dma_start(bv_sb[:], bv_d[:].rearrange("(e o) -> e o", o=1))
            nc.sync.dma_start(tri[:], tri_d[:])
            nc.sync.dma_start(ident[:], ident_d[:])
            wo_sb = const.tile([E, N_STATE], BF16, tag="wo_sb")
            nc.sync.dma_start(wo_sb[:], wo_d[:])
            for b in range(B):
                nc.vector.memset(vn[b][:], 1.0)  # ones columns persist

        # ---------------- stage A: projections for one token group ----------
        def a_units(b, g):
            """Return the list of injectable atoms for token group (b, g)."""
            ts = slice(g * G_TILE, (g + 1) * G_TILE)
            xt = [None]
            vs_box = [None]

            def u_dma():
                xt[0] = xpool.tile([128, N_D, G_TILE], BF16, tag="xt",
                                   name=f"xt{b}_{g}")
                src = xT_d[b, :, ts].rearrange("(c p) t -> p c t", p=128)
                if b == 0 and g == 0:
                    # chunk-split so the first projections start sooner
                    for c2 in range(0, N_D, 2):
                        nc.sync.dma_start(xt[0][:, c2:c2 + 2, :],
                                          src[:, c2:c2 + 2, :])
                else:
                    nc.sync.dma_start(xt[0][:], src)

            def u_q():
                ps = psM.tile([128, 2, Q_TILE], F32, tag="mm")
                for c in range(N_D):
                    for a in range(2):
                        nc.tensor.matmul(
                            ps[:, a, :], wq_sb[:, c, :],
                            xt[0][:, c, a * Q_TILE:(a + 1) * Q_TILE],
                            start=(c == 0), stop=(c == N_D - 1))
                nc.vector.tensor_scalar(qT[b][:, ts],
                                        ps[:].rearrange("p a q -> p (a q)"),
                                        bq_sb[:], SCALE, ALU.add, ALU.mult)

            def u_k():
                ps = psM.tile([128, 2, Q_TILE], F32, tag="mm")
                for c in range(N_D):
                    for a in range(2):
                        nc.tensor.matmul(
                            ps[:, a, :], wk_sb[:, c, :],
                            xt[0][:, c, a * Q_TILE:(a + 1) * Q_TILE],
                            start=(c == 0), stop=(c == N_D - 1))
                nc.vector.tensor_scalar(kT[b][:, ts],
                                        ps[:].rearrange("p a q -> p (a q)"),
                                        SCALE, None, ALU.mult)

            def u_v():
                ps = psM.tile([128, 2, Q_TILE], F32, tag="mm")
                for c in range(N_D):
                    for a in range(2):
                        nc.tensor.matmul(
                            ps[:, a, :], wv_sb[:, c, :],
                            xt[0][:, c, a * Q_TILE:(a + 1) * Q_TILE],
                            start=(c == 0), stop=(c == N_D - 1))
                vs_box[0] = vstage.tile([E, G_TILE], BF16, tag="vs",
                                        name=f"vs{b}_{g}")
                nc.vector.tensor_scalar(vs_box[0][:],
                                        ps[:].rearrange("p a q -> p (a q)"),
                                        bv_sb[:], None, ALU.add)

            def u_vt():
                # PE-transpose the 8 token chunks into a bitcast view of a
                # regular psM slot (bf16 halves it: only cols 0:1024 used),
                # then 2 strided bf16 copies into the vn layout
                tp = psM.tile([128, 2, Q_TILE], F32, tag="mm")
                tpb = tp[:].bitcast(BF16).rearrange(
                    "p a (c t) -> p (a c) t", t=128)
                for t in range(G_TILE // 128):
                    nc.tensor.matmul(tpb[:, t, :],
                                     vs_box[0][:, t * 128:(t + 1) * 128],
                                     ident[:], is_transpose=True)
                cs = slice(g * (G_TILE // 128), (g + 1) * (G_TILE // 128))
                nc.vector.tensor_copy(vn[b][:, cs, 0:HD],
                                      tpb[:, 0:G_TILE // 128, 0:HD])
                nc.vector.tensor_copy(vn[b][:, cs, HD + 1:2 * HD + 1],
                                      tpb[:, 0:G_TILE // 128, HD:2 * HD])

            return [u_dma, u_q, u_k, u_v, u_vt]

        # ---------------- stage B: one (batch, head, half) ------------------
        def b_half(b, h, jp, inject=None, defer_drain=False):
            inject = list(inject or ())
            j0, j1 = 2 * jp, 2 * jp + 1
            n_it = sum(1 for ki in range(n_k)
                       if cls[(ki, j0)] != "skip" or cls[(ki, j1)] != "skip")
            quota = max(1, -(-len(inject) // n_it)) if inject else 0
            hs = slice(h * HD, (h + 1) * HD)
            vslice = slice(h * (HD + 1), (h + 1) * (HD + 1))
            ot = psO.tile([HD + 1, 2, Q_TILE], F32, tag="ot",
                          name=f"ot{b}_{h}_{jp}")

            def emit_ot(ki, pair, wt2f, o):
                for jj, j in ((0, j0), (1, j1)):
                    if pair[jj] == "skip":
                        continue
                    c0 = jj * Q_TILE
                    # diagonal blocks: columns left of the diagonal are all
                    # masked -> stream only [o:] (bf16 moving is 1 cyc/row
                    # at any width); sub-region accumulate is fine on HW
                    sub = pair[jj] == "partial" and o > c0
                    lo = o if sub else c0
                    nc.tensor.matmul(ot[:, jj, lo - c0:Q_TILE],
                                     vn[b][:, ki, vslice],
                                     wt2f[:, lo:c0 + Q_TILE],
                                     start=(ki == 0), stop=(ki == last_ki[j]),
                                     skip_group_check=sub)

            pending = None  # 1-deep SW pipeline: o^T lags scores by a unit
            for ki in range(n_k):
                pair = (cls[(ki, j0)], cls[(ki, j1)])
                if pair == ("skip", "skip"):
                    continue
                for _ in range(quota):
                    if inject:
                        inject.pop(0)()
                sc = psM.tile([128, 2, Q_TILE], F32, tag="mm")
                scf = sc[:].rearrange("p a q -> p (a q)")
                ks = slice(ki * K_CHUNK, (ki + 1) * K_CHUNK)
                def sc_mm(jj, j, part):
                    # left-of-diagonal columns are never read downstream;
                    # skip them when the f32r moving span stays >= 256
                    ol = ki * K_CHUNK - j * Q_TILE if part else 0
                    lo_s = ol if Q_TILE - ol >= 256 else 0
                    qs = slice(j * Q_TILE + lo_s, (j + 1) * Q_TILE)
                    nc.tensor.matmul(sc[:, jj, lo_s:], kT[b][hs, ks],
                                     qT[b][hs, qs], start=True, stop=True)

                if pair[0] == "skip":
                    sc_mm(1, j1, True)
                    o = ki * K_CHUNK - j1 * Q_TILE + Q_TILE  # offset in flat
                else:
                    sc_mm(0, j0, pair[0] == "partial")
                    sc_mm(1, j1, False)
                    o = (ki * K_CHUNK - j0 * Q_TILE) if pair[0] == "partial" else 0
                wt2 = wexp.tile([K_CHUNK, 2, Q_TILE], BF16, tag="wexp")
                wt2f = wt2[:].rearrange("p a q -> p (a q)")
                if pending is not None:
                    emit_ot(*pending)
                nc.scalar.activation(wt2f[:, o:], scf[:, o:], AF.Exp)
                if "partial" in pair:
                    nc.vector.tensor_tensor(wt2f[:, o:o + K_CHUNK],
                                            wt2f[:, o:o + K_CHUNK],
                                            tri[:], ALU.mult)
                pending = (ki, pair, wt2f, o)
            if pending is not None:
                emit_ot(*pending)
            while inject:
                inject.pop(0)()
            # drain: copy PSUM (o rows + denominator row) to SBUF bf16,
            # free the PSUM banks, then normalize from SBUF.  Returned as
            # closures so the caller can inject them into the NEXT half —
            # the bcast round-trip then overlaps instead of head-of-line
            # blocking the DVE queue at the boundary.
            osb = otsb.tile([HD + 1, 2, Q_TILE], BF16, tag="osb")

            def d_copy():
                nc.vector.tensor_copy(osb[:], ot[:])

            def d_norm(jj, j):
                qs = slice(j * Q_TILE, (j + 1) * Q_TILE)
                rd = nrm.tile([1, Q_TILE], BF16, tag="rd")
                with nc.allow_low_precision(reason="softmax denom, 2e-2 gate"):
                    nc.vector.reciprocal(rd[:], osb[HD:HD + 1, jj, :])
                bc = nrm.tile([HD, Q_TILE], BF16, tag="bc")
                rdap = rd[:]
                rd_rep = bass.AP(rdap.tensor, rdap.offset,
                                 [list(rdap.ap[0]), [0, HD], list(rdap.ap[1])])
                nc.sync.dma_start(bc[:], rd_rep)
                nc.vector.tensor_tensor(onT[b][hs, qs], osb[0:HD, jj, :],
                                        bc[:], ALU.mult)

            drain = [d_copy, lambda: d_norm(0, j0), lambda: d_norm(1, j1)]
            if defer_drain:
                return drain
            for d in drain:
                d()
            return []

        # ---------------- stage C: out-proj for (batch, m-chunk, half) ------
        def c_unit(b, m, jp, slot, copy_eng, dma_eng=None):
            ms = slice(m * 128, (m + 1) * 128)
            gs = slice(jp * G_TILE, (jp + 1) * G_TILE)
            pool_, tag = (psC, "yp") if slot == 0 else (psM, "mm")
            yp = pool_.tile([128, 2, Q_TILE], F32, tag=tag)
            ypf = yp[:].rearrange("p a q -> p (a q)")
            for a in range(2):
                qs = slice(jp * G_TILE + a * Q_TILE,
                           jp * G_TILE + (a + 1) * Q_TILE)
                nc.tensor.matmul(yp[:, a, :], wo_sb[:, ms], onT[b][:, qs],
                                 start=True, stop=True)
            ys = yspool.tile([128, G_TILE], BF16, tag="ys")
            # GPSIMD cannot read PSUM, so the drain copy goes on DVE or ACT;
            # the y DMA issues from the otherwise-idle Pool SWDGE queue.
            if copy_eng == "dve":
                nc.vector.tensor_copy(ys[:], ypf)
            else:
                nc.scalar.activation(ys[:], ypf, AF.Copy)
            (dma_eng or nc.gpsimd).dma_start(yT_d[b, ms, gs], ys[:])

        def c_units(b, jp, engs=("dve", "act"), tail=False):
            return [(lambda m_, s_, e_: (lambda: c_unit(
                        b, m_, jp, s_, e_, None)))(
                        m, (m % 2) if tail else 0, engs[m % len(engs)])
                    for m in range(N_STATE // 128)]

        def emit_head(first):
            a0 = a_units(0, 0)
            a1 = a_units(0, 1)
            a0[0]()  # first x DMA right after wq
            if first:
                load_rest_consts()
            a1[0]()
            # both groups' projections first, transposes last: by then the
            # vs staging tiles are long ready, so the PE never waits
            for u in (a0[1], a0[2], a0[3], a1[1], a1[2], a1[3],
                      a0[4], a1[4]):
                u()

        def run_body(last):
            d = b_half(0, 0, 0, inject=a_units(1, 0), defer_drain=True)
            d = b_half(0, 1, 0, inject=d + a_units(1, 1), defer_drain=True)
            cu00 = c_units(0, 0, engs=("dve",))
            d = b_half(0, 0, 1, inject=d + cu00[:4], defer_drain=True)
            d = b_half(0, 1, 1, inject=d + cu00[4:], defer_drain=True)
            cu01 = c_units(0, 1)
            d = b_half(1, 0, 0, inject=d + cu01[:4], defer_drain=True)
            d = b_half(1, 1, 0, inject=d + cu01[4:], defer_drain=True)
            cu10 = c_units(1, 0, engs=("dve",))
            d = b_half(1, 0, 1, inject=d + cu10[:4], defer_drain=True)
            d = b_half(1, 1, 1, inject=d + cu10[4:], defer_drain=True)
            if not last:
                # prefetch the next repeat's batch-0 projections ahead of
                # the final drains and tail C units, so neither the PE
                # queue nor the DVE queue blocks the next repeat's start
                emit_head(False)
            for dd in d:
                dd()
            for u in c_units(1, 1, tail=True):
                u()

        emit_head(True)
        for _rep in range(repeats):
            run_body(_rep == repeats - 1)

    nc.finalize()
    return nc


def shard_inputs(x, mask, Wq, bq, Wk, Wv, bv, Wo):
    """Per-core input dicts (host-side layout prep + slicing only)."""
    bf16 = mybir.dt.np(BF16)
    xT = np.ascontiguousarray(x.transpose(0, 2, 1)).astype(bf16)
    maskT = np.ascontiguousarray(mask.T).astype(np.float32)
    tri = (maskT[:K_CHUNK, :K_CHUNK] > NEG_THRESH).astype(bf16)
    in_maps = []
    for c in range(N_CORES):
        cs = slice(c * E, (c + 1) * E)
        in_maps.append({
            "xT": xT,
            "tri": tri,
            "wq": np.ascontiguousarray(Wq[:, cs]).astype(bf16),
            "wk": np.ascontiguousarray(Wk[:, cs]).astype(bf16),
            "wv": np.ascontiguousarray(Wv[:, cs]).astype(bf16),
            "wo": np.ascontiguousarray(Wo[cs, :]).astype(bf16),
            "bq": np.ascontiguousarray(bq[cs]).astype(np.float32),
            "bv": np.ascontiguousarray(bv[cs]).astype(np.float32),
            "ident": np.eye(128).astype(bf16),
        })
    return in_maps


_NC_CACHE = {}


def _get_nc(B, S, cls_key, cls, repeats=1, only=None):
    key = (B, S, cls_key, repeats, only)
    if key not in _NC_CACHE:
        _NC_CACHE[key] = build_kernel(B, S, cls, repeats=repeats, only=only)
    return _NC_CACHE[key]


def run(x, mask, Wq, bq, Wk, Wv, bv, Wo, bo, trace=False):
    B, S, D = x.shape
    maskT = np.ascontiguousarray(np.asarray(mask).T).astype(np.float32)
    cls = classify_blocks(maskT)
    cls_key = tuple(sorted((k, v) for k, v in cls.items()))
    nc = _get_nc(B, S, hash(cls_key), cls)
    in_maps = shard_inputs(np.asarray(x, np.float32), maskT.T,
                           np.asarray(Wq, np.float32), np.asarray(bq, np.float32),
                           np.asarray(Wk, np.float32), np.asarray(Wv, np.float32),
                           np.asarray(bv, np.float32), np.asarray(Wo, np.float32))
    res = bass_utils.run_bass_kernel_spmd(
        nc, in_maps, core_ids=list(range(N_CORES)), trace=trace)
    acc = np.zeros((B, N_STATE, S), dtype=np.float64)
    for r in res.results:
        acc += r["yT"].astype(np.float64)
    y = acc.transpose(0, 2, 1).astype(np.float32) + np.asarray(bo, np.float32)
    return y, res


def kernel(x, mask, Wq, bq, Wk, Wv, bv, Wo, bo):
    y, _ = run(x, mask, Wq, bq, Wk, Wv, bv, Wo, bo, trace=False)
    return y


def time_run(x, mask, Wq, bq, Wk, Wv, bv, Wo, bo, iters=20, repeats=1,
             only=None):
    """Measure per-iteration device execution time of the SPMD program.

    Mirrors bass2jax.run_bass_via_pjrt's multi-core lowering, but keeps
    inputs device-resident and chains donated output buffers so `iters`
    executions pipeline back-to-back; returns (y, seconds_per_iter).
    """
    import time as _time
    import jax
    from jax.experimental.shard_map import shard_map
    from jax.sharding import Mesh, NamedSharding, PartitionSpec
    from concourse import bass2jax
    from concourse.bass2jax import _bass_exec_p, install_neuronx_cc_hook

    install_neuronx_cc_hook()
    B, S, D = x.shape
    maskT = np.ascontiguousarray(np.asarray(mask).T).astype(np.float32)
    cls = classify_blocks(maskT)
    cls_key = tuple(sorted((k, v) for k, v in cls.items()))
    nc = _get_nc(B, S, hash(cls_key), cls, repeats=repeats, only=only)
    in_maps = shard_inputs(np.asarray(x, np.float32), maskT.T,
                           np.asarray(Wq, np.float32), np.asarray(bq, np.float32),
                           np.asarray(Wk, np.float32), np.asarray(Wv, np.float32),
                           np.asarray(bv, np.float32), np.asarray(Wo, np.float32))

    in_names, out_names, out_avals, zero_outs = [], [], [], []
    partition_name = (nc.partition_id_tensor.name
                      if nc.partition_id_tensor else None)
    for alloc in nc.m.functions[0].allocations:
        if not isinstance(alloc, mybir.MemoryLocationSet):
            continue
        name = alloc.memorylocations[0].name
        if alloc.kind == "ExternalInput":
            if name != partition_name:
                in_names.append(name)
        elif alloc.kind == "ExternalOutput":
            out_names.append(name)
            shape = tuple(alloc.tensor_shape)
            dtype = mybir.dt.np(alloc.dtype)
            out_avals.append((shape, dtype))
            zero_outs.append(np.zeros(shape, dtype))
    n_params = len(in_names)
    n_outs = len(out_names)
    all_in_names = list(in_names) + list(out_names)
    if partition_name is not None:
        all_in_names.append(partition_name)

    def _body(*args):
        operands = list(args)
        if partition_name is not None:
            operands.append(bass2jax.partition_id_tensor())
        outs = _bass_exec_p.bind(
            *operands,
            out_avals=tuple(
                jax.core.ShapedArray(s, d) for s, d in out_avals),
            in_names=tuple(all_in_names),
            out_names=tuple(out_names),
            lowering_input_output_aliases=(),
            sim_require_finite=True,
            sim_require_nnan=True,
            nc=nc,
        )
        return tuple(outs)

    devices = jax.devices()[:N_CORES]
    mesh = Mesh(np.asarray(devices), ("core",))
    spec = PartitionSpec("core")
    donate = tuple(range(n_params, n_params + n_outs))
    sharded = jax.jit(
        shard_map(_body, mesh=mesh, in_specs=(spec,) * (n_params + n_outs),
                  out_specs=(spec,) * n_outs, check_rep=False),
        donate_argnums=donate, keep_unused=True)

    sh = NamedSharding(mesh, spec)
    dev_in = [
        jax.device_put(
            np.concatenate([np.asarray(in_maps[c][nm]) for c in range(N_CORES)],
                           axis=0), sh)
        for nm in in_names
    ]
    out = sharded(*dev_in, *[
        jax.device_put(np.zeros((N_CORES * z.shape[0], *z.shape[1:]), z.dtype),
                       sh) for z in zero_outs])
    jax.block_until_ready(out)  # warmup + compile
    t0 = _time.perf_counter()
    for _ in range(iters):
        out = sharded(*dev_in, *out)
    jax.block_until_ready(out)
    dt = (_time.perf_counter() - t0) / iters

    yT_all = np.asarray(out[out_names.index("yT")])
    acc = np.zeros((B, N_STATE, S), dtype=np.float64)
    for c in range(N_CORES):
        acc += yT_all.reshape(N_CORES, B, N_STATE, S)[c].astype(np.float64)
    y = acc.transpose(0, 2, 1).astype(np.float32) + np.asarray(bo, np.float32)
    return y, dt


# revision 62
# speedup vs baseline: 1.4509x; 1.4509x over previous
"""Multi-head causal attention (Whisper-style) on 8 trn2 NeuronCores.

Sharding: head-parallel (2 of 16 heads per core) + row-parallel output
projection.  Each core receives the full (host-transposed) activations
x^T (bf16), its column slice of Wq/Wk/Wv (+bias slices) and its row
slice of Wo, and produces a full-size partial y^T (bf16).  The host
sums the 8 partials, transposes back, and adds bo.

On-chip layout is fully "transposed" (feature dim on partitions):
  q^T,k^T = W^T-stationary matmuls over x^T -> f32r [128, S] per batch
  scores^T[k,q] per (batch, head) in [128,2,512] PSUM pairs, exp with
  no max-subtraction, softmax denominator via ones-columns in v.
Causal masking needs no mask DMA: blocks fully above the diagonal are
skipped, fully-masked column spans are never exp'd nor streamed through
the o^T matmul (sub-span accumulate), and the single 128x128 diagonal
sub-block is exp'd then multiplied by one shared 0/1 upper-triangular
bf16 tile.  Work from the three stages is interleaved (inject lists +
drains deferred across half boundaries) to keep the PE array busy.

Engine busy per core (cost-model estimate, ~170us wall):
  PE   ~124us  (projections 41 + transposes 3 + attention 63 + out-proj 14)
  ACT  ~94us   (exp + half the out-proj PSUM->SBUF drains)
  DVE  ~81us   (proj finalizes, tri-mults, softmax normalize, drains)
  DMA  ~55us   (x 23 + y 23 + weights/bcast); y issue via Pool SWDGE
"""

import os
import sys
from contextlib import ExitStack

import numpy as np

for _p in ("/root/.axon_site/_ro/trn_rl_repo", "/opt/trn_rl_repo"):
    if os.path.isdir(_p) and _p not in sys.path:
        sys.path.append(_p)

import concourse.bass as bass
import concourse.mybir as mybir
import concourse.tile as tile
from concourse import bacc, bass_utils

F32 = mybir.dt.float32
F32R = mybir.dt.float32r
BF16 = mybir.dt.bfloat16
AF = mybir.ActivationFunctionType
ALU = mybir.AluOpType

N_STATE = 1024
N_HEAD = 16
HD = 64
N_CORES = 8
HEADS_PER_CORE = N_HEAD // N_CORES  # 2
E = HEADS_PER_CORE * HD  # 128 feature columns per core
Q_TILE = 512
K_CHUNK = 128
G_TILE = 1024  # stage A/C token-group (2 q-tiles)
N_D = N_STATE // 128  # 8 contraction chunks for the projections
SCALE = float(HD) ** -0.25
NEG_THRESH = -50.0


def classify_blocks(maskT):
    """Value-driven classification of (k_chunk, q_tile) mask blocks.

    partial blocks additionally verify the shifted-triangular pattern
    (unmasked iff k <= q) that the on-device masking assumes.
    """
    S = maskT.shape[0]
    cls = {}
    for ki in range(S // K_CHUNK):
        for j in range(S // Q_TILE):
            blk = maskT[ki * K_CHUNK:(ki + 1) * K_CHUNK,
                        j * Q_TILE:(j + 1) * Q_TILE]
            if np.all(blk < NEG_THRESH):
                cls[(ki, j)] = "skip"
            elif np.all(blk == 0.0):
                cls[(ki, j)] = "clean"
            else:
                cls[(ki, j)] = "partial"
                k_idx = ki * K_CHUNK + np.arange(K_CHUNK)[:, None]
                q_idx = j * Q_TILE + np.arange(Q_TILE)[None, :]
                ok = (blk > NEG_THRESH) == (k_idx <= q_idx)
                assert ok.all(), f"partial block {(ki, j)} is not causal-triangular"
    return cls


def build_kernel(B, S, cls, repeats=1, only=None):
    """Build the per-core SPMD Bass program (identical on all cores)."""
    n_k = S // K_CHUNK
    n_q = S // Q_TILE
    n_g = S // G_TILE  # token groups per batch

    nc = bacc.Bacc("TRN2", target_bir_lowering=False, debug=False,
                   num_devices=N_CORES)

    xT_d = nc.dram_tensor("xT", [B, N_STATE, S], BF16, kind="ExternalInput")
    tri_d = nc.dram_tensor("tri", [K_CHUNK, K_CHUNK], BF16, kind="ExternalInput")
    wq_d = nc.dram_tensor("wq", [N_STATE, E], BF16, kind="ExternalInput")
    wk_d = nc.dram_tensor("wk", [N_STATE, E], BF16, kind="ExternalInput")
    wv_d = nc.dram_tensor("wv", [N_STATE, E], BF16, kind="ExternalInput")
    wo_d = nc.dram_tensor("wo", [E, N_STATE], BF16, kind="ExternalInput")
    bq_d = nc.dram_tensor("bq", [E], F32, kind="ExternalInput")
    ident_d = nc.dram_tensor("ident", [128, 128], BF16, kind="ExternalInput")
    bv_d = nc.dram_tensor("bv", [E], F32, kind="ExternalInput")
    yT_d = nc.dram_tensor("yT", [B, N_STATE, S], BF16, kind="ExternalOutput")

    # last contributing k-chunk per q-tile (for o^T stop flags)
    last_ki = {j: max(ki for ki in range(n_k) if cls[(ki, j)] != "skip")
               for j in range(n_q)}

    with tile.TileContext(nc) as tc, ExitStack() as ctx:
        const = ctx.enter_context(tc.tile_pool(name="const", bufs=1))
        xpool = ctx.enter_context(tc.tile_pool(name="xpool", bufs=2))
        vstage = ctx.enter_context(tc.tile_pool(name="vstage", bufs=2))
        wexp = ctx.enter_context(tc.tile_pool(name="wexp", bufs=6))
        otsb = ctx.enter_context(tc.tile_pool(name="otsb", bufs=4))
        nrm = ctx.enter_context(tc.tile_pool(name="nrm", bufs=6))
        yspool = ctx.enter_context(tc.tile_pool(name="yspool", bufs=6))
        # PSUM: psM 2x[128,2,512]f32 (4 banks) + psO 1x[65,2,512]f32 (2)
        # + psC 1x[128,2,512]f32 (2) = 8 banks.  C-units get their own slot
        # so a Pool-side backlog can't stall the scores-tile rotation.
        psM = ctx.enter_context(tc.tile_pool(name="psM", bufs=2, space="PSUM"))
        psO = ctx.enter_context(tc.tile_pool(name="psO", bufs=1, space="PSUM"))
        psC = ctx.enter_context(tc.tile_pool(name="psC", bufs=1, space="PSUM"))

        # ---- resident constants / weights (emitted in first-use order) ----
        wq_sb = const.tile([128, N_D, E], BF16, tag="wq_sb")
        wk_sb = const.tile([128, N_D, E], BF16, tag="wk_sb")
        wv_sb = const.tile([128, N_D, E], BF16, tag="wv_sb")
        nc.sync.dma_start(wq_sb[:], wq_d[:].rearrange("(c p) e -> p c e", p=128))
        bq_sb = const.tile([E, 1], F32, tag="bq_sb")
        bv_sb = const.tile([E, 1], F32, tag="bv_sb")
        tri = const.tile([K_CHUNK, K_CHUNK], BF16, tag="tri")
        ident = const.tile([128, 128], BF16, tag="ident")

        qT = [const.tile([E, S], F32R, name=f"qT{b}", tag=f"qT{b}")
              for b in range(B)]
        kT = [const.tile([E, S], F32R, name=f"kT{b}", tag=f"kT{b}")
              for b in range(B)]
        onT = [const.tile([E, S], BF16, name=f"onT{b}", tag=f"onT{b}")
               for b in range(B)]
        # v + ones columns: [h0 64 | 1 | h1 64 | 1] per k-chunk
        vn = [const.tile([128, n_k, 2 * (HD + 1)], BF16, name=f"vn{b}",
                         tag=f"vn{b}") for b in range(B)]

        wo_sb = None

        def load_rest_consts():
            nonlocal wo_sb
            nc.sync.dma_start(wk_sb[:], wk_d[:].rearrange("(c p) e -> p c e", p=128))
            nc.sync.dma_start(wv_sb[:], wv_d[:].rearrange("(c p) e -> p c e", p=128))
            nc.sync.dma_start(bq_sb[:], bq_d[:].rearrange("(e o) -> e o", o=1))
            nc.sync.dma_start(bv_sb[:], bv_d[:].rearrange("(e o) -> e o", o=1))
            nc.sync.dma_start(tri[:], tri_d[:])
            nc.sync.dma_start(ident[:], ident_d[:])
            wo_sb = const.tile([E, N_STATE], BF16, tag="wo_sb")
            nc.sync.dma_start(wo_sb[:], wo_d[:])
            for b in range(B):
                nc.vector.memset(vn[b][:], 1.0)  # ones columns persist

        # ---------------- stage A: projections for one token group ----------
        def a_units(b, g):
            """Return the list of injectable atoms for token group (b, g)."""
            ts = slice(g * G_TILE, (g + 1) * G_TILE)
            xt = [None]
            vs_box = [None]

            def u_dma():
                xt[0] = xpool.tile([128, N_D, G_TILE], BF16, tag="xt",
                                   name=f"xt{b}_{g}")
                src = xT_d[b, :, ts].rearrange("(c p) t -> p c t", p=128)
                if b == 0 and g == 0:
                    # chunk-split so the first projections start sooner
                    for c2 in range(0, N_D, 2):
                        nc.sync.dma_start(xt[0][:, c2:c2 + 2, :],
                                          src[:, c2:c2 + 2, :])
                else:
                    nc.sync.dma_start(xt[0][:], src)

            def u_q():
                ps = psM.tile([128, 2, Q_TILE], F32, tag="mm")
                for c in range(N_D):
                    for a in range(2):
                        nc.tensor.matmul(
                            ps[:, a, :], wq_sb[:, c, :],
                            xt[0][:, c, a * Q_TILE:(a + 1) * Q_TILE],
                            start=(c == 0), stop=(c == N_D - 1))
                nc.vector.tensor_scalar(qT[b][:, ts],
                                        ps[:].rearrange("p a q -> p (a q)"),
                                        bq_sb[:], SCALE, ALU.add, ALU.mult)

            def u_k():
                ps = psM.tile([128, 2, Q_TILE], F32, tag="mm")
                for c in range(N_D):
                    for a in range(2):
                        nc.tensor.matmul(
                            ps[:, a, :], wk_sb[:, c, :],
                            xt[0][:, c, a * Q_TILE:(a + 1) * Q_TILE],
                            start=(c == 0), stop=(c == N_D - 1))
                nc.vector.tensor_scalar(kT[b][:, ts],
                                        ps[:].rearrange("p a q -> p (a q)"),
                                        SCALE, None, ALU.mult)

            def u_v():
                ps = psM.tile([128, 2, Q_TILE], F32, tag="mm")
                for c in range(N_D):
                    for a in range(2):
                        nc.tensor.matmul(
                            ps[:, a, :], wv_sb[:, c, :],
                            xt[0][:, c, a * Q_TILE:(a + 1) * Q_TILE],
                            start=(c == 0), stop=(c == N_D - 1))
                vs_box[0] = vstage.tile([E, G_TILE], BF16, tag="vs",
                                        name=f"vs{b}_{g}")
                nc.vector.tensor_scalar(vs_box[0][:],
                                        ps[:].rearrange("p a q -> p (a q)"),
                                        bv_sb[:], None, ALU.add)

            def u_vt():
                # PE-transpose the 8 token chunks into a bitcast view of a
                # regular psM slot (bf16 halves it: only cols 0:1024 used),
                # then 2 strided bf16 copies into the vn layout
                tp = psM.tile([128, 2, Q_TILE], F32, tag="mm")
                tpb = tp[:].bitcast(BF16).rearrange(
                    "p a (c t) -> p (a c) t", t=128)
                for t in range(G_TILE // 128):
                    nc.tensor.matmul(tpb[:, t, :],
                                     vs_box[0][:, t * 128:(t + 1) * 128],
                                     ident[:], is_transpose=True)
                cs = slice(g * (G_TILE // 128), (g + 1) * (G_TILE // 128))
                nc.vector.tensor_copy(vn[b][:, cs, 0:HD],
                                      tpb[:, 0:G_TILE // 128, 0:HD])
                nc.vector.tensor_copy(vn[b][:, cs, HD + 1:2 * HD + 1],
                                      tpb[:, 0:G_TILE // 128, HD:2 * HD])

            return [u_dma, u_q, u_k, u_v, u_vt]

        # ---------------- stage B: one (batch, head, half) ------------------
        def b_half(b, h, jp, inject=None, defer_drain=False):
            inject = list(inject or ())
            j0, j1 = 2 * jp, 2 * jp + 1
            n_it = sum(1 for ki in range(n_k)
                       if cls[(ki, j0)] != "skip" or cls[(ki, j1)] != "skip")
            quota = max(1, -(-len(inject) // n_it)) if inject else 0
            hs = slice(h * HD, (h + 1) * HD)
            vslice = slice(h * (HD + 1), (h + 1) * (HD + 1))
            ot = psO.tile([HD + 1, 2, Q_TILE], F32, tag="ot",
                          name=f"ot{b}_{h}_{jp}")

            def emit_ot(ki, pair, wt2f, o):
                for jj, j in ((0, j0), (1, j1)):
                    if pair[jj] == "skip":
                        continue
                    c0 = jj * Q_TILE
                    # diagonal blocks: columns left of the diagonal are all
                    # masked -> stream only [o:] (bf16 moving is 1 cyc/row
                    # at any width); sub-region accumulate is fine on HW
                    sub = pair[jj] == "partial" and o > c0
                    lo = o if sub else c0
                    nc.tensor.matmul(ot[:, jj, lo - c0:Q_TILE],
                                     vn[b][:, ki, vslice],
                                     wt2f[:, lo:c0 + Q_TILE],
                                     start=(ki == 0), stop=(ki == last_ki[j]),
                                     skip_group_check=sub)

            pending = None  # 1-deep SW pipeline: o^T lags scores by a unit
            for ki in range(n_k):
                pair = (cls[(ki, j0)], cls[(ki, j1)])
                if pair == ("skip", "skip"):
                    continue
                for _ in range(quota):
                    if inject:
                        inject.pop(0)()
                sc = psM.tile([128, 2, Q_TILE], F32, tag="mm")
                scf = sc[:].rearrange("p a q -> p (a q)")
                ks = slice(ki * K_CHUNK, (ki + 1) * K_CHUNK)
                def sc_mm(jj, j, part):
                    # left-of-diagonal columns are never read downstream;
                    # skip them when the f32r moving span stays >= 256
                    ol = ki * K_CHUNK - j * Q_TILE if part else 0
                    lo_s = ol if Q_TILE - ol >= 256 else 0
                    qs = slice(j * Q_TILE + lo_s, (j + 1) * Q_TILE)
                    nc.tensor.matmul(sc[:, jj, lo_s:], kT[b][hs, ks],
                                     qT[b][hs, qs], start=True, stop=True)

                if pair[0] == "skip":
                    sc_mm(1, j1, True)
                    o = ki * K_CHUNK - j1 * Q_TILE + Q_TILE  # offset in flat
                else:
                    sc_mm(0, j0, pair[0] == "partial")
                    sc_mm(1, j1, False)
                    o = (ki * K_CHUNK - j0 * Q_TILE) if pair[0] == "partial" else 0
                wt2 = wexp.tile([K_CHUNK, 2, Q_TILE], BF16, tag="wexp")
                wt2f = wt2[:].rearrange("p a q -> p (a q)")
                if pending is not None:
                    emit_ot(*pending)
                nc.scalar.activation(wt2f[:, o:], scf[:, o:], AF.Exp)
                if "partial" in pair:
                    nc.vector.tensor_tensor(wt2f[:, o:o + K_CHUNK],
                                            wt2f[:, o:o + K_CHUNK],
                                            tri[:], ALU.mult)
                pending = (ki, pair, wt2f, o)
            if pending is not None:
                emit_ot(*pending)
            while inject:
                inject.pop(0)()
            # drain: copy PSUM (o rows + denominator row) to SBUF bf16,
            # free the PSUM banks, then normalize from SBUF.  Returned as
            # closures so the caller can inject them into the NEXT half —
            # the bcast round-trip then overlaps instead of head-of-line
            # blocking the DVE queue at the boundary.
            osb = otsb.tile([HD + 1, 2, Q_TILE], BF16, tag="osb")

            def d_copy():
                nc.vector.tensor_copy(osb[:], ot[:])

            def d_norm(jj, j):
                qs = slice(j * Q_TILE, (j + 1) * Q_TILE)
                rd = nrm.tile([1, Q_TILE], BF16, tag="rd")
                with nc.allow_low_precision(reason="softmax denom, 2e-2 gate"):
                    nc.vector.reciprocal(rd[:], osb[HD:HD + 1, jj, :])
                bc = nrm.tile([HD, Q_TILE], BF16, tag="bc")
                rdap = rd[:]
                rd_rep = bass.AP(rdap.tensor, rdap.offset,
                                 [list(rdap.ap[0]), [0, HD], list(rdap.ap[1])])
                nc.sync.dma_start(bc[:], rd_rep)
                nc.vector.tensor_tensor(onT[b][hs, qs], osb[0:HD, jj, :],
                                        bc[:], ALU.mult)

            drain = [d_copy, lambda: d_norm(0, j0), lambda: d_norm(1, j1)]
            if defer_drain:
                return drain
            for d in drain:
                d()
            return []

        # ---------------- stage C: out-proj for (batch, m-chunk, half) ------
        def c_unit(b, m, jp, slot, copy_eng, dma_eng=None):
            ms = slice(m * 128, (m + 1) * 128)
            gs = slice(jp * G_TILE, (jp + 1) * G_TILE)
            pool_, tag = (psC, "yp") if slot == 0 else (psM, "mm")
            yp = pool_.tile([128, 2, Q_TILE], F32, tag=tag)
            ypf = yp[:].rearrange("p a q -> p (a q)")
            for a in range(2):
                qs = slice(jp * G_TILE + a * Q_TILE,
                           jp * G_TILE + (a + 1) * Q_TILE)
                nc.tensor.matmul(yp[:, a, :], wo_sb[:, ms], onT[b][:, qs],
                                 start=True, stop=True)
            ys = yspool.tile([128, G_TILE], BF16, tag="ys")
            # GPSIMD cannot read PSUM, so the drain copy goes on DVE or ACT;
            # the y DMA issues from the otherwise-idle Pool SWDGE queue.
            if copy_eng == "dve":
                nc.vector.tensor_copy(ys[:], ypf)
            else:
                nc.scalar.activation(ys[:], ypf, AF.Copy)
            (dma_eng or nc.gpsimd).dma_start(yT_d[b, ms, gs], ys[:])

        def c_units(b, jp, engs=("dve", "act"), tail=False):
            return [(lambda m_, s_, e_: (lambda: c_unit(
                        b, m_, jp, s_, e_, None)))(
                        m, (m % 2) if tail else 0, engs[m % len(engs)])
                    for m in range(N_STATE // 128)]

        def emit_head(first):
            a0 = a_units(0, 0)
            a1 = a_units(0, 1)
            a0[0]()  # first x DMA right after wq
            if first:
                load_rest_consts()
            a1[0]()
            # both groups' projections first, transposes last: by then the
            # vs staging tiles are long ready, so the PE never waits
            for u in (a0[1], a0[2], a0[3], a1[1], a1[2], a1[3],
                      a0[4], a1[4]):
                u()

        def run_body(last):
            d = b_half(0, 0, 0, inject=a_units(1, 0), defer_drain=True)
            d = b_half(0, 1, 0, inject=d + a_units(1, 1), defer_drain=True)
            cu00 = c_units(0, 0, engs=("dve",))
            d = b_half(0, 0, 1, inject=d + cu00[:4], defer_drain=True)
            d = b_half(0, 1, 1, inject=d + cu00[4:], defer_drain=True)
            cu01 = c_units(0, 1)
            d = b_half(1, 0, 0, inject=d + cu01[:4], defer_drain=True)
            d = b_half(1, 1, 0, inject=d + cu01[4:], defer_drain=True)
            cu10 = c_units(1, 0, engs=("dve",))
            d = b_half(1, 0, 1, inject=d + cu10[:4], defer_drain=True)
            d = b_half(1, 1, 1, inject=d + cu10[4:], defer_drain=True)
            if not last:
                # prefetch the next repeat's batch-0 projections ahead of
                # the final drains and tail C units, so neither the PE
                # queue nor the DVE queue blocks the next repeat's start
                emit_head(False)
            for dd in d:
                dd()
            for u in c_units(1, 1, tail=True):
                u()

        emit_head(True)
        for _rep in range(repeats):
            run_body(_rep == repeats - 1)

    nc.finalize()
    return nc


def shard_inputs(x, mask, Wq, bq, Wk, Wv, bv, Wo):
    """Per-core input dicts (host-side layout prep + slicing only)."""
    bf16 = mybir.dt.np(BF16)
    xT = np.ascontiguousarray(x.transpose(0, 2, 1)).astype(bf16)
    maskT = np.ascontiguousarray(mask.T).astype(np.float32)
    tri = (maskT[:K_CHUNK, :K_CHUNK] > NEG_THRESH).astype(bf16)
    in_maps = []
    for c in range(N_CORES):
        cs = slice(c * E, (c + 1) * E)
        in_maps.append({
            "xT": xT,
            "tri": tri,
            "wq": np.ascontiguousarray(Wq[:, cs]).astype(bf16),
            "wk": np.ascontiguousarray(Wk[:, cs]).astype(bf16),
            "wv": np.ascontiguousarray(Wv[:, cs]).astype(bf16),
            "wo": np.ascontiguousarray(Wo[cs, :]).astype(bf16),
            "bq": np.ascontiguousarray(bq[cs]).astype(np.float32),
            "bv": np.ascontiguousarray(bv[cs]).astype(np.float32),
            "ident": np.eye(128).astype(bf16),
        })
    return in_maps


_NC_CACHE = {}


def _get_nc(B, S, cls_key, cls, repeats=1, only=None):
    key = (B, S, cls_key, repeats, only)
    if key not in _NC_CACHE:
        _NC_CACHE[key] = build_kernel(B, S, cls, repeats=repeats, only=only)
    return _NC_CACHE[key]


def run(x, mask, Wq, bq, Wk, Wv, bv, Wo, bo, trace=False):
    B, S, D = x.shape
    maskT = np.ascontiguousarray(np.asarray(mask).T).astype(np.float32)
    cls = classify_blocks(maskT)
    cls_key = tuple(sorted((k, v) for k, v in cls.items()))
    nc = _get_nc(B, S, hash(cls_key), cls)
    in_maps = shard_inputs(np.asarray(x, np.float32), maskT.T,
                           np.asarray(Wq, np.float32), np.asarray(bq, np.float32),
                           np.asarray(Wk, np.float32), np.asarray(Wv, np.float32),
                           np.asarray(bv, np.float32), np.asarray(Wo, np.float32))
    res = bass_utils.run_bass_kernel_spmd(
        nc, in_maps, core_ids=list(range(N_CORES)), trace=trace)
    acc = np.zeros((B, N_STATE, S), dtype=np.float64)
    for r in res.results:
        acc += r["yT"].astype(np.float64)
    y = acc.transpose(0, 2, 1).astype(np.float32) + np.asarray(bo, np.float32)
    return y, res


def kernel(x, mask, Wq, bq, Wk, Wv, bv, Wo, bo):
    y, _ = run(x, mask, Wq, bq, Wk, Wv, bv, Wo, bo, trace=False)
    return y


def time_run(x, mask, Wq, bq, Wk, Wv, bv, Wo, bo, iters=20, repeats=1,
             only=None):
    """Measure per-iteration device execution time of the SPMD program.

    Mirrors bass2jax.run_bass_via_pjrt's multi-core lowering, but keeps
    inputs device-resident and chains donated output buffers so `iters`
    executions pipeline back-to-back; returns (y, seconds_per_iter).
    """
    import time as _time
    import jax
    from jax.experimental.shard_map import shard_map
    from jax.sharding import Mesh, NamedSharding, PartitionSpec
    from concourse import bass2jax
    from concourse.bass2jax import _bass_exec_p, install_neuronx_cc_hook

    install_neuronx_cc_hook()
    B, S, D = x.shape
    maskT = np.ascontiguousarray(np.asarray(mask).T).astype(np.float32)
    cls = classify_blocks(maskT)
    cls_key = tuple(sorted((k, v) for k, v in cls.items()))
    nc = _get_nc(B, S, hash(cls_key), cls, repeats=repeats, only=only)
    in_maps = shard_inputs(np.asarray(x, np.float32), maskT.T,
                           np.asarray(Wq, np.float32), np.asarray(bq, np.float32),
                           np.asarray(Wk, np.float32), np.asarray(Wv, np.float32),
                           np.asarray(bv, np.float32), np.asarray(Wo, np.float32))

    in_names, out_names, out_avals, zero_outs = [], [], [], []
    partition_name = (nc.partition_id_tensor.name
                      if nc.partition_id_tensor else None)
    for alloc in nc.m.functions[0].allocations:
        if not isinstance(alloc, mybir.MemoryLocationSet):
            continue
        name = alloc.memorylocations[0].name
        if alloc.kind == "ExternalInput":
            if name != partition_name:
                in_names.append(name)
        elif alloc.kind == "ExternalOutput":
            out_names.append(name)
            shape = tuple(alloc.tensor_shape)
            dtype = mybir.dt.np(alloc.dtype)
            out_avals.append((shape, dtype))
            zero_outs.append(np.zeros(shape, dtype))
    n_params = len(in_names)
    n_outs = len(out_names)
    all_in_names = list(in_names) + list(out_names)
    if partition_name is not None:
        all_in_names.append(partition_name)

    def _body(*args):
        operands = list(args)
        if partition_name is not None:
            operands.append(bass2jax.partition_id_tensor())
        outs = _bass_exec_p.bind(
            *operands,
            out_avals=tuple(
                jax.core.ShapedArray(s, d) for s, d in out_avals),
            in_names=tuple(all_in_names),
            out_names=tuple(out_names),
            lowering_input_output_aliases=(),
            sim_require_finite=True,
            sim_require_nnan=True,
            nc=nc,
        )
        return tuple(outs)

    devices = jax.devices()[:N_CORES]
    mesh = Mesh(np.asarray(devices), ("core",))
    spec = PartitionSpec("core")
    donate = tuple(range(n_params, n_params + n_outs))
    sharded = jax.jit(
        shard_map(_body, mesh=mesh, in_specs=(spec,) * (n_params + n_outs),
                  out_specs=(spec,) * n_outs, check_rep=False),
        donate_argnums=donate, keep_unused=True)

    sh = NamedSharding(mesh, spec)
    dev_in = [
        jax.device_put(
            np.concatenate([np.asarray(in_maps[c][nm]) for c in range(N_CORES)],
                           axis=0), sh)
        for nm in in_names
    ]
    out = sharded(*dev_in, *[
        jax.device_put(np.zeros((N_CORES * z.shape[0], *z.shape[1:]), z.dtype),
                       sh) for z in zero_outs])
    jax.block_until_ready(out)  # warmup + compile
    t0 = _time.perf_counter()
    for _ in range(iters):
        out = sharded(*dev_in, *out)
    jax.block_until_ready(out)
    dt = (_time.perf_counter() - t0) / iters

    yT_all = np.asarray(out[out_names.index("yT")])
    acc = np.zeros((B, N_STATE, S), dtype=np.float64)
    for c in range(N_CORES):
        acc += yT_all.reshape(N_CORES, B, N_STATE, S)[c].astype(np.float64)
    y = acc.transpose(0, 2, 1).astype(np.float32) + np.asarray(bo, np.float32)
    return y, dt
